# revision 2
# baseline (speedup 1.0000x reference)
"""JiT/DiT transformer block (adaLN + attention + SwiGLU) on 8 TRN2 NeuronCores.

Data-parallel over batch: core i computes batch element i end-to-end; no
collectives. Activations are kept "transposed" on device ([channel, seq]) so
per-channel modulation/bias are per-partition scalars; attention scores are
produced directly in [k, q] layout (softmax denominator via a ones-row
appended to V inside the AV matmul). Matmuls run bf16 with fp32 PSUM
accumulation; the residual stream stays fp32.
"""

import sys

sys.path.insert(0, "/opt/trn_rl_repo")

import numpy as np
import ml_dtypes

import concourse.bacc as bacc
import concourse.bass as bass
import concourse.mybir as mybir
from concourse.tile import TileContext
from concourse.bass_utils import run_bass_kernel_spmd

F32 = mybir.dt.float32
BF16 = mybir.dt.bfloat16
AF = mybir.ActivationFunctionType
ALU = mybir.AluOpType

B, S, D, H = 8, 1024, 1024, 16
HD = D // H  # 64
INNER = 2730
INNER_P = 2816  # 22*128
P = 128
NT = 8
NKT12 = INNER_P // P  # 22
EPS = 1e-6

_CACHE = {}


def _to_pmaj(v):
    return np.ascontiguousarray(v.reshape(-1, P).T)


def _rope_perm():
    ev = np.arange(0, HD, 2)
    od = np.arange(1, HD, 2)
    perm = np.concatenate([ev, od])
    partner = np.concatenate([od, ev])
    return perm, partner


def _prep_weights(inp):
    """Host-side layout/dtype prep (reordering/padding only, no math)."""
    perm, partner = _rope_perm()
    chperm = (np.arange(D).reshape(H, HD) [:, perm]).reshape(-1)

    w_qkv, b_qkv = inp["w_qkv"], inp["b_qkv"]
    wq = w_qkv[:, 0:D][:, chperm]
    wk = w_qkv[:, D : 2 * D][:, chperm]
    wv = w_qkv[:, 2 * D :]
    bq = b_qkv[0:D][chperm]
    bk = b_qkv[D : 2 * D][chperm]
    bv = b_qkv[2 * D :]
    wv_ext = np.zeros((D, H * 65), np.float32)
    bv_ext = np.zeros((H * 65,), np.float32)
    for h in range(H):
        wv_ext[:, h * 65 : h * 65 + 64] = wv[:, h * 64 : (h + 1) * 64]
        bv_ext[h * 65 : h * 65 + 64] = bv[h * 64 : (h + 1) * 64]
        bv_ext[h * 65 + 64] = 1.0
    wqkv_cat = np.concatenate([wq, wk, wv_ext], axis=1)  # [D, 3088]

    w12, b12 = inp["w12"], inp["b12"]
    w12p = np.zeros((D, 2 * INNER_P), np.float32)
    b12p = np.zeros((2 * INNER_P,), np.float32)
    w12p[:, :INNER] = w12[:, :INNER]
    w12p[:, INNER_P : INNER_P + INNER] = w12[:, INNER:]
    b12p[:INNER] = b12[:INNER]
    b12p[INNER_P : INNER_P + INNER] = b12[INNER:]
    w3p = np.zeros((INNER_P, D), np.float32)
    w3p[:INNER] = inp["w3"]

    # rope tiles [128, S]: two stacked 64-row head-local blocks
    sign = np.where(np.arange(HD) < HD // 2, -1.0, 1.0).astype(np.float32)
    cos, sin = inp["rope_cos"], inp["rope_sin"]

    def rope_tiles(scale_vec):
        c64 = cos[:, perm].T * scale_vec[perm][:, None]
        s64 = (sin[:, perm].T * sign[:, None]) * scale_vec[partner][:, None]
        return (
            np.concatenate([c64, c64], 0).astype(np.float32),
            np.concatenate([s64, s64], 0).astype(np.float32),
        )

    cq, sq = rope_tiles(inp["qn_scale"])
    ck, sk = rope_tiles(inp["kn_scale"])

    E2 = np.zeros((2, P), np.float32)
    E2[0, 0:64] = 1.0
    E2[1, 64:128] = 1.0
    e65 = np.zeros((65, 64), np.float32)
    e65[64, :] = 1.0
    bo2 = np.zeros((P, 2), np.float32)
    bo2[0:64, 0] = 1.0
    bo2[64:128, 1] = 1.0

    bqk_T = np.stack(
        [bq.reshape(NT, P)[m] for m in range(NT)]
        + [bk.reshape(NT, P)[m] for m in range(NT)],
        axis=1,
    )

    return {
        "wqkv": wqkv_cat, "wproj": inp["w_proj"], "w12p": w12p, "w3p": w3p,
        "wada": inp["w_ada"], "bqk_T": bqk_T, "bv_ext": bv_ext[None, :],
        "b12T": _to_pmaj(b12p), "bprojT": _to_pmaj(inp["b_proj"]),
        "b3T": _to_pmaj(inp["b3"]), "n1T": _to_pmaj(inp["norm1_scale"]),
        "n2T": _to_pmaj(inp["norm2_scale"]), "b_ada": inp["b_ada"][None, :],
        "E2": E2, "e65": e65, "bo2": bo2, "ones1": np.ones((1, P), np.float32),
        "ident": np.eye(P, dtype=np.float32),
        "cos2q": cq, "sin2q": sq, "cos2k": ck, "sin2k": sk,
    }


BF16_NAMES = {
    "wqkv", "wproj", "w12p", "w3p", "wada", "bv_ext", "E2", "e65", "bo2", "ones1",
    "cos2q", "sin2q", "cos2k", "sin2k",
}


def build_bass():
    nc = bacc.Bacc("TRN2", target_bir_lowering=False, debug=False, num_devices=8)

    def par(name, shape, dt, out=False):
        return nc.declare_dram_parameter(name, list(shape), dt, isOutput=out)

    d = {
        "x": par("x", [S, D], F32),
        "cT": par("cT", [P, NT], F32),
        "wqkv": par("wqkv", [D, 2 * D + H * 65], BF16),
        "wproj": par("wproj", [D, D], BF16),
        "w12p": par("w12p", [D, 2 * INNER_P], BF16),
        "w3p": par("w3p", [INNER_P, D], BF16),
        "wada": par("wada", [D, 6 * D], BF16),
        "bqk_T": par("bqk_T", [P, 16], F32),
        "bv_ext": par("bv_ext", [1, H * 65], BF16),
        "b12T": par("b12T", [P, 2 * NKT12], F32),
        "bprojT": par("bprojT", [P, NT], F32),
        "b3T": par("b3T", [P, NT], F32),
        "n1T": par("n1T", [P, NT], F32),
        "n2T": par("n2T", [P, NT], F32),
        "b_ada": par("b_ada", [1, 6 * D], F32),
        "E2": par("E2", [2, P], BF16),
        "e65": par("e65", [65, 64], BF16),
        "bo2": par("bo2", [P, 2], BF16),
        "ones1": par("ones1", [1, P], BF16),
        "ident": par("ident", [P, P], F32),
        "cos2q": par("cos2q", [P, S], BF16),
        "sin2q": par("sin2q", [P, S], BF16),
        "cos2k": par("cos2k", [P, S], BF16),
        "sin2k": par("sin2k", [P, S], BF16),
        "out": par("out", [S, D], F32, out=True),
    }
    mods_dram = nc.dram_tensor("mods_scratch", [1, 6 * D], F32)
    kss_dram = nc.dram_tensor("kss_scratch", [H, S], F32)

    with TileContext(nc) as tc:
        _body(nc, tc, d, mods_dram, kss_dram)
    nc.compile()
    return nc


def _body(nc, tc, d, mods_dram, kss_dram):
    from contextlib import ExitStack

    with ExitStack() as ctx:
        const = ctx.enter_context(tc.tile_pool(name="const", bufs=1))
        persist = ctx.enter_context(tc.tile_pool(name="persist", bufs=1))
        small = ctx.enter_context(tc.tile_pool(name="small", bufs=1))
        scratch = ctx.enter_context(tc.tile_pool(name="scratch", bufs=2))
        psum = ctx.enter_context(tc.tile_pool(name="psum", bufs=6, space="PSUM"))

        def load_const(key, shape, dt, pool=None):
            t = (pool or const).tile(list(shape), dt, tag=key, name=key + "_sb")
            nc.sync.dma_start(out=t[:], in_=d[key][:])
            return t

        cT = load_const("cT", [P, NT], F32)
        bqkT = load_const("bqk_T", [P, 16], F32)
        bv = load_const("bv_ext", [1, H * 65], BF16)
        b12T = load_const("b12T", [P, 2 * NKT12], F32)
        bprojT = load_const("bprojT", [P, NT], F32)
        b3T = load_const("b3T", [P, NT], F32)
        n1T = load_const("n1T", [P, NT], F32)
        n2T = load_const("n2T", [P, NT], F32)
        bo2 = load_const("bo2", [P, 2], BF16)
        e65 = load_const("e65", [65, 64], BF16)
        ones1 = load_const("ones1", [1, P], BF16)
        ident = load_const("ident", [P, P], F32)
        ones128 = const.tile([P, P], BF16, tag="ones128", name="ones128")
        nc.vector.memset(ones128[:], 1.0)
        eps1 = const.tile([P, 1], F32, tag="eps1", name="eps1")
        nc.vector.memset(eps1[:], EPS)
        epsk = const.tile([P, 1], F32, tag="epsk", name="epsk")
        nc.vector.memset(epsk[:], HD * EPS)

        # residual stream lives here, updated in place
        xT = persist.tile([P, NT, S], F32, tag="bigf32", name="xT")
        invb = persist.tile([P, S], F32, tag="invb", name="invb")
        invrk8 = small.tile([P, NT, H], F32, name="invrk8", padded_shape=[P, NT, H + 1])

        def rms_invb(zT):
            # invb[:, ch*512:...] = 1/sqrt(mean_d z^2 + eps) (rows identical)
            for ch in range(2):
                ms = None
                for dt in range(NT):
                    sq = scratch.tile([P, 512], BF16, tag="sqd", name="sqd")
                    nc.vector.tensor_mul(
                        sq[:],
                        zT[:, dt, ch * 512 : (ch + 1) * 512],
                        zT[:, dt, ch * 512 : (ch + 1) * 512],
                    )
                    if dt == 0:
                        ms = psum.tile([P, 512], F32, tag="ps", name="ps_ms")
                    nc.tensor.matmul(
                        ms[:], ones128[:], sq[:],
                        start=(dt == 0), stop=(dt == NT - 1),
                    )
                rms = scratch.tile([P, 512], F32, tag="rms", name="rms")
                nc.scalar.activation(rms[:], ms[:], AF.Sqrt, bias=eps1[:], scale=1.0 / D)
                nc.vector.reciprocal_approx_fast(
                    invb[:, ch * 512 : (ch + 1) * 512], rms[:]
                )

        def modulate(zT, dstT, aa, sh):
            for dt in range(NT):
                tmp = scratch.tile([P, S], F32, tag="htmp", name="htmp")
                nc.vector.tensor_mul(tmp[:], zT[:, dt, :], invb[:])
                nc.vector.tensor_scalar(
                    dstT[:, dt, :], tmp[:], aa[:, dt : dt + 1], sh[:, dt : dt + 1],
                    op0=ALU.mult, op1=ALU.add,
                )

        # ======= Phases B-E =======
        with ExitStack() as actx:
            ho = actx.enter_context(tc.tile_pool(name="ho", bufs=1))
            hT = ho.tile([P, NT, S], BF16, tag="hT", name="hT")
            ohat = ho.tile([P, NT, S], BF16, tag="ohat", name="ohat")

            # ---- Phase B ----
            with tc.tile_pool(name="xin_pool", bufs=3) as xin_pool:
                for st in range(NT):
                    xin = xin_pool.tile([P, D], F32, tag="xin", name="xin")
                    nc.sync.dma_start(out=xin[:], in_=d["x"][st * P : (st + 1) * P, :])
                    for g4 in range(2):
                        pt = psum.tile([P, 512], F32, tag="ps", name="ps_tr")
                        for j in range(4):
                            dt = g4 * 4 + j
                            nc.tensor.transpose(
                                pt[:, j * P : (j + 1) * P],
                                xin[:, dt * P : (dt + 1) * P],
                                ident[:],
                            )
                        for j in range(4):
                            dt = g4 * 4 + j
                            nc.scalar.activation(
                                xT[:, dt, st * P : (st + 1) * P],
                                pt[:, j * P : (j + 1) * P],
                                AF.Copy,
                            )

            rms_invb(xT)

            # ============ Phase A: mods ============
            cT_silu = small.tile([P, NT], F32, name="cT_silu")
            nc.scalar.activation(cT_silu[:], cT[:], AF.Silu)
            cT_bf = small.tile([P, NT], BF16, name="cT_bf")
            nc.vector.tensor_copy(cT_bf[:], cT_silu[:])

            with tc.tile_pool(name="ada_sc", bufs=2) as ada_sc, tc.tile_pool(
                name="wada_pool", bufs=2
            ) as wada_pool:
                for n in range(12):
                    ps = psum.tile([1, 512], F32, tag="ps", name="ps_ada")
                    wt = wada_pool.tile([P, NT, 512], BF16, tag="wada", name="wada_t")
                    nc.sync.dma_start(
                        out=wt[:],
                        in_=d["wada"][:, n * 512 : (n + 1) * 512].rearrange(
                            "(kt p) c -> p kt c", p=P
                        ),
                    )
                    for kt in range(NT):
                        nc.tensor.matmul(
                            ps[:], cT_bf[:, kt : kt + 1], wt[:, kt, :],
                            start=(kt == 0), stop=(kt == NT - 1),
                        )
                    bch = ada_sc.tile([1, 512], F32, tag="bch", name="bada_ch")
                    nc.sync.dma_start(out=bch[:], in_=d["b_ada"][:, n * 512 : (n + 1) * 512])
                    mch = ada_sc.tile([1, 512], F32, tag="mch", name="mods_ch")
                    nc.vector.tensor_add(mch[:], ps[:], bch[:])
                    nc.sync.dma_start(
                        out=mods_dram[:, n * 512 : (n + 1) * 512], in_=mch[:]
                    )
            modsT = small.tile([P, 48], F32, name="modsT")
            nc.sync.dma_start(
                out=modsT[:], in_=mods_dram.ap()[0, :].rearrange("(t p) -> p t", p=P)
            )
            a1 = small.tile([P, NT], F32, name="a1")
            nc.vector.tensor_scalar_add(a1[:], modsT[:, 8:16], 1.0)
            nc.vector.tensor_mul(a1[:], a1[:], n1T[:])
            sh1 = modsT[:, 0:8]
            g1 = modsT[:, 16:24]
            g1b = small.tile([P, NT], F32, name="g1b")
            nc.vector.tensor_mul(g1b[:], g1, bprojT[:])
            a2 = small.tile([P, NT], F32, name="a2")
            nc.vector.tensor_scalar_add(a2[:], modsT[:, 32:40], 1.0)
            nc.vector.tensor_mul(a2[:], a2[:], n2T[:])
            sh2 = modsT[:, 24:32]
            g2 = modsT[:, 40:48]
            g2b3 = small.tile([P, NT], F32, name="g2b3")
            nc.vector.tensor_mul(g2b3[:], g2, b3T[:])


            modulate(xT, hT, a1, sh1)

            # ---- Phases C + D in a scoped block ----
            with ExitStack() as cctx:
                qk = cctx.enter_context(tc.tile_pool(name="qk", bufs=1))
                qhat = qk.tile([P, NT, S], BF16, tag="qhat", name="qhat")
                khat = qk.tile([P, NT, S], BF16, tag="khat", name="khat")
                v_sb = qk.tile([P, NT, H * 65], BF16, tag="v", name="v_sb")

                with ExitStack() as qctx:
                    ropec = qctx.enter_context(tc.tile_pool(name="ropec", bufs=1))
                    qkn = qctx.enter_context(tc.tile_pool(name="qkn", bufs=1))
                    wqk_pool = qctx.enter_context(tc.tile_pool(name="wqk_pool", bufs=3))
                    rope_sc = qctx.enter_context(tc.tile_pool(name="rope_sc", bufs=2))

                    cos2q = load_const("cos2q", [P, S], BF16, pool=ropec)
                    sin2q = load_const("sin2q", [P, S], BF16, pool=ropec)
                    cos2k = load_const("cos2k", [P, S], BF16, pool=ropec)
                    sin2k = load_const("sin2k", [P, S], BF16, pool=ropec)
                    E2 = load_const("E2", [2, P], BF16, pool=ropec)

                    for m in range(16):
                        isq = m < NT
                        mk = m if isq else m - NT
                        wt = wqk_pool.tile([P, NT, P], BF16, tag="wqk", name="wqk_t")
                        nc.sync.dma_start(
                            out=wt[:],
                            in_=d["wqkv"][:, m * P : (m + 1) * P].rearrange(
                                "(kt p) c -> p kt c", p=P
                            ),
                        )
                        raw = rope_sc.tile([P, S], BF16, tag="raw", name="qk_raw")
                        for sch in range(2):
                            ps = psum.tile([P, 512], F32, tag="ps", name="ps_qkv")
                            for kt in range(NT):
                                nc.tensor.matmul(
                                    ps[:], wt[:, kt, :],
                                    hT[:, kt, sch * 512 : (sch + 1) * 512],
                                    start=(kt == 0), stop=(kt == NT - 1),
                                )
                            nc.vector.tensor_scalar_add(
                                raw[:, sch * 512 : (sch + 1) * 512], ps[:],
                                bqkT[:, m : m + 1],
                            )
                            sqs = scratch.tile([P, 512], BF16, tag="sqd", name="sqs")
                            nc.vector.tensor_mul(
                                sqs[:],
                                raw[:, sch * 512 : (sch + 1) * 512],
                                raw[:, sch * 512 : (sch + 1) * 512],
                            )
                            ss = psum.tile([2, 512], F32, tag="ps", name="ps_ss")
                            nc.tensor.matmul(ss[:], bo2[:], sqs[:], start=True, stop=True)
                            if isq:
                                if sch == 0:
                                    qt = qkn.tile(
                                        [2, S], F32, tag="qstage", name="qstage", bufs=2
                                    )
                                nc.scalar.activation(
                                    qt[:, sch * 512 : (sch + 1) * 512],
                                    ss[:], AF.Copy,
                                )
                            else:
                                if sch == 0:
                                    kstage = qkn.tile(
                                        [2, S], F32, tag="kstage", name="kstage", bufs=2
                                    )
                                nc.scalar.activation(
                                    kstage[:, sch * 512 : (sch + 1) * 512], ss[:], AF.Copy
                                )
                                nc.sync.dma_start(
                                    out=kss_dram[
                                        2 * mk : 2 * mk + 2,
                                        sch * 512 : (sch + 1) * 512,
                                    ],
                                    in_=kstage[:, sch * 512 : (sch + 1) * 512],
                                )
                        rot = rope_sc.tile([P, S], BF16, tag="rot", name="rot", bufs=2)
                        for blk in range(4):
                            b0 = blk * 32
                            srcb = b0 + (32 if blk % 2 == 0 else -32)
                            nc.gpsimd.dma_start(
                                out=rot[b0 : b0 + 32, :], in_=raw[srcb : srcb + 32, :]
                            )
                        t1 = rope_sc.tile([P, S], BF16, tag="t1", name="rope_t1", bufs=2)
                        t2 = rope_sc.tile([P, S], BF16, tag="t2", name="rope_t2", bufs=2)
                        nc.vector.tensor_mul(t1[:], raw[:], cos2q[:] if isq else cos2k[:])
                        nc.vector.tensor_mul(t2[:], rot[:], sin2q[:] if isq else sin2k[:])
                        nc.vector.tensor_add(
                            (qhat if isq else khat)[:, mk, :], t1[:], t2[:]
                        )
                        if isq:
                            # inverse-rms of this q pair, folded into qhat now
                            nc.scalar.activation(
                                qt[:], qt[:], AF.Sqrt, bias=eps1[0:2, :],
                                scale=1.0 / HD,
                            )
                            nc.vector.reciprocal_approx_fast(qt[:], qt[:])
                            qbf = qkn.tile([2, S], BF16, tag="qbf", name="qbf", bufs=2)
                            nc.vector.tensor_copy(qbf[:], qt[:])
                            for sch in range(2):
                                pe = psum.tile([P, 512], F32, tag="ps", name="ps_erq")
                                nc.tensor.matmul(
                                    pe[:], E2[:],
                                    qbf[:, sch * 512 : (sch + 1) * 512],
                                    start=True, stop=True,
                                )
                                nc.vector.tensor_mul(
                                    qhat[:, mk, sch * 512 : (sch + 1) * 512],
                                    qhat[:, mk, sch * 512 : (sch + 1) * 512], pe[:],
                                )

                    # q inverse-rms per m-tile pair
                    kssT = qkn.tile([P, NT, H], F32, name="kssT", padded_shape=[P, NT, H + 1])
                    for kt in range(NT):
                        nc.sync.dma_start(
                            out=kssT[:, kt, :],
                            in_=kss_dram.ap()[:, kt * P : (kt + 1) * P].rearrange(
                                "h p -> p h"
                            ),
                        )
                    for kt in range(NT):
                        nc.scalar.activation(
                            kssT[:, kt, :], kssT[:, kt, :], AF.Sqrt,
                            bias=epsk[:], scale=1.0,
                        )
                        nc.vector.reciprocal_approx_fast(
                            invrk8[:, kt, :], kssT[:, kt, :]
                        )

                    # q inverse-rms handled inline above

                    # v
                    with tc.tile_pool(name="wv_pool", bufs=2) as wv_pool:
                        for nch in range(4):
                            c0 = nch * 260
                            wt = wv_pool.tile([P, NT, 260], BF16, tag="wv", name="wv_t")
                            nc.sync.dma_start(
                                out=wt[:],
                                in_=d["wqkv"][
                                    :, 2 * D + c0 : 2 * D + c0 + 260
                                ].rearrange("(kt p) c -> p kt c", p=P),
                            )
                            for st in range(NT):
                                ps = psum.tile([P, 260], F32, tag="ps", name="ps_v")
                                for kt in range(NT):
                                    nc.tensor.matmul(
                                        ps[:], hT[:, kt, st * P : (st + 1) * P],
                                        wt[:, kt, :],
                                        start=(kt == 0), stop=False,
                                    )
                                nc.tensor.matmul(
                                    ps[:], ones1[:], bv[:, c0 : c0 + 260],
                                    start=False, stop=True,
                                )
                                nc.vector.tensor_copy(
                                    v_sb[:, st, c0 : c0 + 260], ps[:]
                                )

                # ---- Phase D: attention ----
                with tc.tile_pool(name="ppool", bufs=3) as ppool, tc.tile_pool(
                    name="avp", bufs=2, space="PSUM"
                ) as avp, tc.tile_pool(name="att_sc", bufs=2) as att_sc:

                    def qk_exp(h, qch):
                        mk, hh = h // 2, h % 2
                        rb = 64 * hh
                        pT = ppool.tile([P, NT, 512], BF16, tag="pT", name="pT")
                        for kt in range(NT):
                            ps_s = psum.tile([P, 512], F32, tag="ps", name="ps_s")
                            nc.tensor.matmul(
                                ps_s[:],
                                khat[rb : rb + 64, mk, kt * P : (kt + 1) * P],
                                qhat[rb : rb + 64, mk, qch * 512 : (qch + 1) * 512],
                                start=True, stop=True,
                            )
                            nc.scalar.activation(
                                pT[:, kt, :], ps_s[:], AF.Exp,
                                scale=invrk8[:, kt, h : h + 1],
                            )
                        return pT

                    def av_div(h, qch, pT):
                        mk, hh = h // 2, h % 2
                        rb = 64 * hh
                        ps_av = avp.tile([65, 512], F32, tag="ps_av", name="ps_av")
                        for kt in range(NT):
                            nc.tensor.matmul(
                                ps_av[:], v_sb[:, kt, h * 65 : h * 65 + 65],
                                pT[:, kt, :],
                                start=(kt == 0), stop=(kt == NT - 1),
                            )
                        o65 = att_sc.tile([65, 512], F32, tag="o65", name="o65")
                        nc.vector.tensor_copy(o65[:], ps_av[:])
                        o65b = att_sc.tile([65, 512], BF16, tag="o65b", name="o65b")
                        nc.vector.tensor_copy(o65b[:], o65[:])
                        pb = psum.tile([64, 512], F32, tag="ps", name="ps_bc")
                        nc.tensor.matmul(pb[:], e65[:], o65b[:], start=True, stop=True)
                        rb64 = att_sc.tile([64, 512], F32, tag="rb64", name="rb64")
                        nc.vector.reciprocal_approx_fast(rb64[:], pb[:])
                        ob = att_sc.tile([64, 512], BF16, tag="ob", name="ob")
                        nc.vector.tensor_mul(ob[:], o65[0:64, :], rb64[:])
                        nc.sync.dma_start(
                            out=ohat[rb : rb + 64, mk, qch * 512 : (qch + 1) * 512],
                            in_=ob[:],
                        )

                    prev = None
                    for h in range(H):
                        for qch in range(2):
                            pT = qk_exp(h, qch)
                            if prev is not None:
                                av_div(*prev)
                            prev = (h, qch, pT)
                    av_div(*prev)

            # ---- Phase E: proj + residual 1 (in place on xT) ----
            with tc.tile_pool(name="wproj_pool", bufs=3) as wproj_pool:
                for dt in range(NT):
                    wt = wproj_pool.tile([P, NT, P], BF16, tag="wproj", name="wproj_t")
                    nc.sync.dma_start(
                        out=wt[:],
                        in_=d["wproj"][:, dt * P : (dt + 1) * P].rearrange(
                            "(kt p) c -> p kt c", p=P
                        ),
                    )
                    for qch in range(2):
                        ps = psum.tile([P, 512], F32, tag="ps", name="ps_proj")
                        for kt in range(NT):
                            nc.tensor.matmul(
                                ps[:], wt[:, kt, :],
                                ohat[:, kt, qch * 512 : (qch + 1) * 512],
                                start=(kt == 0), stop=(kt == NT - 1),
                            )
                        nc.vector.affine_then_add(
                            xT[:, dt, qch * 512 : (qch + 1) * 512],
                            ps[:], xT[:, dt, qch * 512 : (qch + 1) * 512],
                            scale=g1[:, dt : dt + 1], bias=g1b[:, dt : dt + 1],
                        )

        # ======= Phases F-H =======
        with ExitStack() as mctx:
            mlp = mctx.enter_context(tc.tile_pool(name="mlp", bufs=1))

            rms_invb(xT)

            h2T = mlp.tile([P, NT, S], BF16, tag="h2T", name="h2T")
            modulate(xT, h2T, a2, sh2)

            gg = mlp.tile([P, NKT12, S], BF16, tag="gg", name="gg")
            with tc.tile_pool(name="w12_pool", bufs=3) as w12_pool, tc.tile_pool(
                name="mlp_sc", bufs=2
            ) as mlp_sc:
                for j in range(NKT12):
                    outs = []
                    for part in range(2):
                        m = j + part * NKT12
                        wt = w12_pool.tile([P, NT, P], BF16, tag="w12", name="w12_t")
                        nc.sync.dma_start(
                            out=wt[:],
                            in_=d["w12p"][:, m * P : (m + 1) * P].rearrange(
                                "(kt p) c -> p kt c", p=P
                            ),
                        )
                        o = mlp_sc.tile([P, S], BF16, tag=f"mlp{part}", name=f"mlp{part}")
                        for sch in range(2):
                            ps = psum.tile([P, 512], F32, tag="ps", name="ps_mlp")
                            for kt in range(NT):
                                nc.tensor.matmul(
                                    ps[:], wt[:, kt, :],
                                    h2T[:, kt, sch * 512 : (sch + 1) * 512],
                                    start=(kt == 0), stop=(kt == NT - 1),
                                )
                            nc.scalar.activation(
                                o[:, sch * 512 : (sch + 1) * 512], ps[:],
                                AF.Silu if part == 0 else AF.Identity,
                                bias=b12T[:, m : m + 1],
                            )
                        outs.append(o)
                    nc.vector.tensor_mul(gg[:, j, :], outs[0][:], outs[1][:])

            # w3 + residual 2 (in place on xT)
            with tc.tile_pool(name="w3_pool", bufs=2) as w3_pool:
                for dt in range(NT):
                    wt = w3_pool.tile([P, NKT12, P], BF16, tag="w3", name="w3_t")
                    nc.sync.dma_start(
                        out=wt[:],
                        in_=d["w3p"][:, dt * P : (dt + 1) * P].rearrange(
                            "(kt p) c -> p kt c", p=P
                        ),
                    )
                    for qch in range(2):
                        ps = psum.tile([P, 512], F32, tag="ps", name="ps_w3")
                        for kt in range(NKT12):
                            nc.tensor.matmul(
                                ps[:], wt[:, kt, :],
                                gg[:, kt, qch * 512 : (qch + 1) * 512],
                                start=(kt == 0), stop=(kt == NKT12 - 1),
                            )
                        nc.vector.affine_then_add(
                            xT[:, dt, qch * 512 : (qch + 1) * 512],
                            ps[:], xT[:, dt, qch * 512 : (qch + 1) * 512],
                            scale=g2[:, dt : dt + 1], bias=g2b3[:, dt : dt + 1],
                        )

            # ---- Phase H ----
            with tc.tile_pool(name="yout", bufs=3) as ypool:
                for st in range(NT):
                    y = ypool.tile([P, D], F32, tag="y", name="y")
                    for g4 in range(2):
                        pt = psum.tile([P, 512], F32, tag="ps", name="ps_tr2")
                        for j in range(4):
                            dt = g4 * 4 + j
                            nc.tensor.transpose(
                                pt[:, j * P : (j + 1) * P],
                                xT[:, dt, st * P : (st + 1) * P],
                                ident[:],
                            )
                        for j in range(4):
                            dt = g4 * 4 + j
                            nc.scalar.activation(
                                y[:, dt * P : (dt + 1) * P],
                                pt[:, j * P : (j + 1) * P],
                                AF.Copy,
                            )
                    nc.sync.dma_start(out=d["out"][st * P : (st + 1) * P, :], in_=y[:])


def kernel(**inputs):
    inputs = {k: np.asarray(v) for k, v in inputs.items()}
    if "nc" not in _CACHE:
        _CACHE["nc"] = build_bass()
    nc = _CACHE["nc"]

    consts = _prep_weights(inputs)
    base = {}
    for k, v in consts.items():
        if k in BF16_NAMES:
            base[k] = np.ascontiguousarray(v).astype(ml_dtypes.bfloat16)
        else:
            base[k] = np.ascontiguousarray(v).astype(np.float32)

    in_maps = []
    for core in range(B):
        m = dict(base)
        m["x"] = np.ascontiguousarray(inputs["x"][core]).astype(np.float32)
        m["cT"] = _to_pmaj(inputs["c"][core]).astype(np.float32)
        in_maps.append(m)

    res = run_bass_kernel_spmd(
        nc, in_maps, core_ids=list(range(B)), **_CACHE.get("run_kwargs", {})
    )
    _CACHE["last_results"] = res
    return np.stack([res.results[i]["out"] for i in range(B)], axis=0)


if __name__ == "__main__":
    build_bass()
    print("built ok")



# revision 15
# speedup vs baseline: 1.0199x; 1.0199x over previous
"""JiT/DiT transformer block (adaLN + attention + SwiGLU) on 8 TRN2 NeuronCores.

Data-parallel over batch: core i computes batch element i end-to-end; no
collectives. Activations are kept "transposed" on device ([channel, seq]) so
per-channel modulation/bias are per-partition scalars; attention scores are
produced directly in [k, q] layout (softmax denominator via a ones-row
appended to V inside the AV matmul). Matmuls run bf16 with fp32 PSUM
accumulation; the residual stream stays fp32.
"""

import sys

sys.path.insert(0, "/opt/trn_rl_repo")

import numpy as np
import ml_dtypes

import concourse.bacc as bacc
import concourse.bass as bass
import concourse.mybir as mybir
from concourse.tile import TileContext
from concourse.bass_utils import run_bass_kernel_spmd

F32 = mybir.dt.float32
BF16 = mybir.dt.bfloat16
AF = mybir.ActivationFunctionType
ALU = mybir.AluOpType

B, S, D, H = 8, 1024, 1024, 16
HD = D // H  # 64
INNER = 2730
INNER_P = 2816  # 22*128
P = 128
NT = 8
NKT12 = INNER_P // P  # 22
EPS = 1e-6

_CACHE = {}


def _to_pmaj(v):
    return np.ascontiguousarray(v.reshape(-1, P).T)


def _rope_perm():
    ev = np.arange(0, HD, 2)
    od = np.arange(1, HD, 2)
    perm = np.concatenate([ev, od])
    partner = np.concatenate([od, ev])
    return perm, partner


def _prep_weights(inp):
    """Host-side layout/dtype prep (reordering/padding only, no math)."""
    perm, partner = _rope_perm()
    chperm = (np.arange(D).reshape(H, HD) [:, perm]).reshape(-1)

    w_qkv, b_qkv = inp["w_qkv"], inp["b_qkv"]
    wq = w_qkv[:, 0:D][:, chperm]
    wk = w_qkv[:, D : 2 * D][:, chperm]
    wv = w_qkv[:, 2 * D :]
    bq = b_qkv[0:D][chperm]
    bk = b_qkv[D : 2 * D][chperm]
    bv = b_qkv[2 * D :]
    wv_ext = np.zeros((D, H * 65), np.float32)
    bv_ext = np.zeros((H * 65,), np.float32)
    for h in range(H):
        wv_ext[:, h * 65 : h * 65 + 64] = wv[:, h * 64 : (h + 1) * 64]
        bv_ext[h * 65 : h * 65 + 64] = bv[h * 64 : (h + 1) * 64]
        bv_ext[h * 65 + 64] = 1.0
    wqkv_cat = np.concatenate([wq, wk, wv_ext], axis=1)  # [D, 3088]

    w12, b12 = inp["w12"], inp["b12"]
    w12p = np.zeros((D, 2 * INNER_P), np.float32)
    b12p = np.zeros((2 * INNER_P,), np.float32)
    w12p[:, :INNER] = w12[:, :INNER]
    w12p[:, INNER_P : INNER_P + INNER] = w12[:, INNER:]
    b12p[:INNER] = b12[:INNER]
    b12p[INNER_P : INNER_P + INNER] = b12[INNER:]
    w3p = np.zeros((INNER_P, D), np.float32)
    w3p[:INNER] = inp["w3"]

    # rope tiles [128, S]: two stacked 64-row head-local blocks
    sign = np.where(np.arange(HD) < HD // 2, -1.0, 1.0).astype(np.float32)
    cos, sin = inp["rope_cos"], inp["rope_sin"]

    def rope_tiles(scale_vec):
        c64 = cos[:, perm].T * scale_vec[perm][:, None]
        s64 = (sin[:, perm].T * sign[:, None]) * scale_vec[partner][:, None]
        return (
            np.concatenate([c64, c64], 0).astype(np.float32),
            np.concatenate([s64, s64], 0).astype(np.float32),
        )

    cq, sq = rope_tiles(inp["qn_scale"])
    ck, sk = rope_tiles(inp["kn_scale"])

    E2 = np.zeros((2, P), np.float32)
    E2[0, 0:64] = 1.0
    E2[1, 64:128] = 1.0
    e65 = np.zeros((65, 64), np.float32)
    e65[64, :] = 1.0
    bo2 = np.zeros((P, 2), np.float32)
    bo2[0:64, 0] = 1.0
    bo2[64:128, 1] = 1.0

    bqk_T = np.stack(
        [bq.reshape(NT, P)[m] for m in range(NT)]
        + [bk.reshape(NT, P)[m] for m in range(NT)],
        axis=1,
    )

    # [m, p, kt, c] prepack: weight W[K, M] consumed as m-tiles of [P, kt, C]
    def pack(w, C):
        K, M = w.shape
        return np.ascontiguousarray(
            w.reshape(K // P, P, M // C, C).transpose(2, 1, 0, 3)
        )

    return {
        "wqkv16": pack(wqkv_cat[:, : 2 * D], P),
        "wv4": pack(wqkv_cat[:, 2 * D :], H * 65 // 4),
        "wproj8": pack(inp["w_proj"], P),
        "w1244": pack(w12p, P),
        "w38": pack(w3p, P),
        "wada12": pack(inp["w_ada"], 512),
        "bqk_T": bqk_T, "bv_ext": bv_ext[None, :],
        "b12T": _to_pmaj(b12p), "bprojT": _to_pmaj(inp["b_proj"]),
        "b3T": _to_pmaj(inp["b3"]), "n1T": _to_pmaj(inp["norm1_scale"]),
        "n2T": _to_pmaj(inp["norm2_scale"]), "b_ada": inp["b_ada"][None, :],
        "E2": E2, "e65": e65, "bo2": bo2, "ones1": np.ones((1, P), np.float32),
        "ident": np.eye(P, dtype=np.float32),
        "cos2q": cq, "sin2q": sq, "cos2k": ck, "sin2k": sk,
    }


BF16_NAMES = {
    "wqkv16", "wv4", "wproj8", "w1244", "w38", "wada12",
    "bv_ext", "E2", "e65", "bo2", "ones1",
    "cos2q", "sin2q", "cos2k", "sin2k",
}


def build_bass():
    nc = bacc.Bacc("TRN2", target_bir_lowering=False, debug=False, num_devices=8)

    def par(name, shape, dt, out=False):
        return nc.declare_dram_parameter(name, list(shape), dt, isOutput=out)

    d = {
        "x": par("x", [S, D], F32),
        "cT": par("cT", [P, NT], F32),
        "wqkv16": par("wqkv16", [16, P, NT, P], BF16),
        "wv4": par("wv4", [4, P, NT, H * 65 // 4], BF16),
        "wproj8": par("wproj8", [NT, P, NT, P], BF16),
        "w1244": par("w1244", [2 * NKT12, P, NT, P], BF16),
        "w38": par("w38", [NT, P, NKT12, P], BF16),
        "wada12": par("wada12", [12, P, NT, 512], BF16),
        "bqk_T": par("bqk_T", [P, 16], F32),
        "bv_ext": par("bv_ext", [1, H * 65], BF16),
        "b12T": par("b12T", [P, 2 * NKT12], F32),
        "bprojT": par("bprojT", [P, NT], F32),
        "b3T": par("b3T", [P, NT], F32),
        "n1T": par("n1T", [P, NT], F32),
        "n2T": par("n2T", [P, NT], F32),
        "b_ada": par("b_ada", [1, 6 * D], F32),
        "E2": par("E2", [2, P], BF16),
        "e65": par("e65", [65, 64], BF16),
        "bo2": par("bo2", [P, 2], BF16),
        "ones1": par("ones1", [1, P], BF16),
        "ident": par("ident", [P, P], F32),
        "cos2q": par("cos2q", [P, S], BF16),
        "sin2q": par("sin2q", [P, S], BF16),
        "cos2k": par("cos2k", [P, S], BF16),
        "sin2k": par("sin2k", [P, S], BF16),
        "out": par("out", [S, D], F32, out=True),
    }
    mods_dram = nc.dram_tensor("mods_scratch", [1, 6 * D], F32)
    kss_dram = nc.dram_tensor("kss_scratch", [H, S], F32)

    with TileContext(nc) as tc:
        _body(nc, tc, d, mods_dram, kss_dram)
    nc.compile()
    return nc


def _body(nc, tc, d, mods_dram, kss_dram):
    from contextlib import ExitStack

    with ExitStack() as ctx:
        const = ctx.enter_context(tc.tile_pool(name="const", bufs=1))
        persist = ctx.enter_context(tc.tile_pool(name="persist", bufs=1))
        small = ctx.enter_context(tc.tile_pool(name="small", bufs=1))
        scratch = ctx.enter_context(tc.tile_pool(name="scratch", bufs=2))
        psum = ctx.enter_context(tc.tile_pool(name="psum", bufs=6, space="PSUM"))

        def load_const(key, shape, dt, pool=None):
            t = (pool or const).tile(list(shape), dt, tag=key, name=key + "_sb")
            nc.sync.dma_start(out=t[:], in_=d[key][:])
            return t

        cT = load_const("cT", [P, NT], F32)
        bqkT = load_const("bqk_T", [P, 16], F32)
        bv = load_const("bv_ext", [1, H * 65], BF16)
        b12T = load_const("b12T", [P, 2 * NKT12], F32)
        bprojT = load_const("bprojT", [P, NT], F32)
        b3T = load_const("b3T", [P, NT], F32)
        n1T = load_const("n1T", [P, NT], F32)
        n2T = load_const("n2T", [P, NT], F32)
        bo2 = load_const("bo2", [P, 2], BF16)
        e65 = load_const("e65", [65, 64], BF16)
        ones1 = load_const("ones1", [1, P], BF16)
        ident = load_const("ident", [P, P], F32)
        ones128 = const.tile([P, P], BF16, tag="ones128", name="ones128")
        nc.vector.memset(ones128[:], 1.0)
        eps1 = const.tile([P, 1], F32, tag="eps1", name="eps1")
        nc.vector.memset(eps1[:], EPS)
        epsk = const.tile([P, 1], F32, tag="epsk", name="epsk")
        nc.vector.memset(epsk[:], HD * EPS)

        # residual stream lives here, updated in place
        xT = persist.tile([P, NT, S], F32, tag="bigf32", name="xT")
        invb = persist.tile([P, S], F32, tag="invb", name="invb")
        invrk8 = small.tile([P, NT, H], F32, name="invrk8", padded_shape=[P, NT, H + 1])

        def rms_invb(zT):
            # invb[:, ch*512:...] = 1/sqrt(mean_d z^2 + eps) (rows identical)
            for ch in range(2):
                ms = None
                for dt in range(NT):
                    sq = scratch.tile([P, 512], BF16, tag="sqd", name="sqd")
                    nc.vector.tensor_mul(
                        sq[:],
                        zT[:, dt, ch * 512 : (ch + 1) * 512],
                        zT[:, dt, ch * 512 : (ch + 1) * 512],
                    )
                    if dt == 0:
                        ms = psum.tile([P, 512], F32, tag="ps", name="ps_ms")
                    nc.tensor.matmul(
                        ms[:], ones128[:], sq[:],
                        start=(dt == 0), stop=(dt == NT - 1),
                    )
                rms = scratch.tile([P, 512], F32, tag="rms", name="rms")
                nc.scalar.activation(rms[:], ms[:], AF.Sqrt, bias=eps1[:], scale=1.0 / D)
                nc.vector.reciprocal_approx_fast(
                    invb[:, ch * 512 : (ch + 1) * 512], rms[:]
                )

        def modulate(zT, dstT, aa, sh):
            for dt in range(NT):
                tmp = scratch.tile([P, S], F32, tag="htmp", name="htmp")
                nc.vector.tensor_mul(tmp[:], zT[:, dt, :], invb[:])
                nc.vector.tensor_scalar(
                    dstT[:, dt, :], tmp[:], aa[:, dt : dt + 1], sh[:, dt : dt + 1],
                    op0=ALU.mult, op1=ALU.add,
                )

        # ======= Phases B-E =======
        with ExitStack() as actx:
            ho = actx.enter_context(tc.tile_pool(name="ho", bufs=1))
            hT = ho.tile([P, NT, S], BF16, tag="hT", name="hT")
            ohat = ho.tile([P, NT, S], BF16, tag="ohat", name="ohat")

            # ---- Phase B ----
            with tc.tile_pool(name="xin_pool", bufs=3) as xin_pool:
                for st in range(NT):
                    xin = xin_pool.tile([P, D], F32, tag="xin", name="xin")
                    nc.sync.dma_start(
                        out=xin[:, 0:512], in_=d["x"][st * P : (st + 1) * P, 0:512]
                    )
                    nc.sync.dma_start(
                        out=xin[:, 512:D], in_=d["x"][st * P : (st + 1) * P, 512:D]
                    )
                    for g4 in range(2):
                        pt = psum.tile([P, 512], F32, tag="ps", name="ps_tr")
                        for j in range(4):
                            dt = g4 * 4 + j
                            nc.tensor.transpose(
                                pt[:, j * P : (j + 1) * P],
                                xin[:, dt * P : (dt + 1) * P],
                                ident[:],
                            )
                        for j in range(4):
                            dt = g4 * 4 + j
                            nc.scalar.activation(
                                xT[:, dt, st * P : (st + 1) * P],
                                pt[:, j * P : (j + 1) * P],
                                AF.Copy,
                            )

            rms_invb(xT)

            # ============ Phase A: mods ============
            cT_silu = small.tile([P, NT], F32, name="cT_silu")
            nc.scalar.activation(cT_silu[:], cT[:], AF.Silu)
            cT_bf = small.tile([P, NT], BF16, name="cT_bf")
            nc.vector.tensor_copy(cT_bf[:], cT_silu[:])

            with tc.tile_pool(name="ada_sc", bufs=2) as ada_sc, tc.tile_pool(
                name="wada_pool", bufs=3
            ) as wada_pool:
                for n in range(12):
                    ps = psum.tile([1, 512], F32, tag="ps", name="ps_ada")
                    wt = wada_pool.tile([P, NT, 512], BF16, tag="wada", name="wada_t")
                    nc.sync.dma_start(out=wt[:, 0:4, :], in_=d["wada12"][n, :, 0:4, :])
                    nc.sync.dma_start(out=wt[:, 4:8, :], in_=d["wada12"][n, :, 4:8, :])
                    for kt in range(NT):
                        nc.tensor.matmul(
                            ps[:], cT_bf[:, kt : kt + 1], wt[:, kt, :],
                            start=(kt == 0), stop=(kt == NT - 1),
                        )
                    bch = ada_sc.tile([1, 512], F32, tag="bch", name="bada_ch")
                    nc.sync.dma_start(out=bch[:], in_=d["b_ada"][:, n * 512 : (n + 1) * 512])
                    mch = ada_sc.tile([1, 512], F32, tag="mch", name="mods_ch")
                    nc.vector.tensor_add(mch[:], ps[:], bch[:])
                    nc.sync.dma_start(
                        out=mods_dram[:, n * 512 : (n + 1) * 512], in_=mch[:]
                    )
            modsT = small.tile([P, 48], F32, name="modsT")
            nc.sync.dma_start(
                out=modsT[:], in_=mods_dram.ap()[0, :].rearrange("(t p) -> p t", p=P)
            )
            a1 = small.tile([P, NT], F32, name="a1")
            nc.vector.tensor_scalar_add(a1[:], modsT[:, 8:16], 1.0)
            nc.vector.tensor_mul(a1[:], a1[:], n1T[:])
            sh1 = modsT[:, 0:8]
            g1 = modsT[:, 16:24]
            g1b = small.tile([P, NT], F32, name="g1b")
            nc.vector.tensor_mul(g1b[:], g1, bprojT[:])
            a2 = small.tile([P, NT], F32, name="a2")
            nc.vector.tensor_scalar_add(a2[:], modsT[:, 32:40], 1.0)
            nc.vector.tensor_mul(a2[:], a2[:], n2T[:])
            sh2 = modsT[:, 24:32]
            g2 = modsT[:, 40:48]
            g2b3 = small.tile([P, NT], F32, name="g2b3")
            nc.vector.tensor_mul(g2b3[:], g2, b3T[:])


            modulate(xT, hT, a1, sh1)

            # ---- Phases C + D in a scoped block ----
            with ExitStack() as cctx:
                qk = cctx.enter_context(tc.tile_pool(name="qk", bufs=1))
                qhat = qk.tile([P, NT, S], BF16, tag="qhat", name="qhat")
                khat = qk.tile([P, NT, S], BF16, tag="khat", name="khat")
                v_sb = qk.tile([P, NT, H * 65], BF16, tag="v", name="v_sb")

                with ExitStack() as qctx:
                    ropec = qctx.enter_context(tc.tile_pool(name="ropec", bufs=1))
                    qkn = qctx.enter_context(tc.tile_pool(name="qkn", bufs=1))
                    wqk_pool = qctx.enter_context(tc.tile_pool(name="wqk_pool", bufs=4))
                    rope_sc = qctx.enter_context(tc.tile_pool(name="rope_sc", bufs=2))

                    cos2q = load_const("cos2q", [P, S], BF16, pool=ropec)
                    sin2q = load_const("sin2q", [P, S], BF16, pool=ropec)
                    cos2k = load_const("cos2k", [P, S], BF16, pool=ropec)
                    sin2k = load_const("sin2k", [P, S], BF16, pool=ropec)
                    E2 = load_const("E2", [2, P], BF16, pool=ropec)

                    for m in range(16):
                        isq = m < NT
                        mk = m if isq else m - NT
                        wt = wqk_pool.tile([P, NT, P], BF16, tag="wqk", name="wqk_t")
                        nc.sync.dma_start(out=wt[:, 0:4, :], in_=d["wqkv16"][m, :, 0:4, :])
                        nc.sync.dma_start(out=wt[:, 4:8, :], in_=d["wqkv16"][m, :, 4:8, :])
                        raw = rope_sc.tile([P, S], BF16, tag="raw", name="qk_raw")
                        for sch in range(2):
                            ps = psum.tile([P, 512], F32, tag="ps", name="ps_qkv")
                            for kt in range(NT):
                                nc.tensor.matmul(
                                    ps[:], wt[:, kt, :],
                                    hT[:, kt, sch * 512 : (sch + 1) * 512],
                                    start=(kt == 0), stop=(kt == NT - 1),
                                )
                            nc.vector.tensor_scalar_add(
                                raw[:, sch * 512 : (sch + 1) * 512], ps[:],
                                bqkT[:, m : m + 1],
                            )
                            sqs = scratch.tile([P, 512], BF16, tag="sqd", name="sqs")
                            nc.vector.tensor_mul(
                                sqs[:],
                                raw[:, sch * 512 : (sch + 1) * 512],
                                raw[:, sch * 512 : (sch + 1) * 512],
                            )
                            ss = psum.tile([2, 512], F32, tag="ps", name="ps_ss")
                            nc.tensor.matmul(ss[:], bo2[:], sqs[:], start=True, stop=True)
                            if isq:
                                if sch == 0:
                                    qt = qkn.tile(
                                        [2, S], F32, tag="qstage", name="qstage", bufs=2
                                    )
                                nc.scalar.activation(
                                    qt[:, sch * 512 : (sch + 1) * 512],
                                    ss[:], AF.Copy,
                                )
                            else:
                                if sch == 0:
                                    kstage = qkn.tile(
                                        [2, S], F32, tag="kstage", name="kstage", bufs=2
                                    )
                                nc.scalar.activation(
                                    kstage[:, sch * 512 : (sch + 1) * 512], ss[:], AF.Copy
                                )
                                nc.sync.dma_start(
                                    out=kss_dram[
                                        2 * mk : 2 * mk + 2,
                                        sch * 512 : (sch + 1) * 512,
                                    ],
                                    in_=kstage[:, sch * 512 : (sch + 1) * 512],
                                )
                        rot = rope_sc.tile([P, S], BF16, tag="rot", name="rot", bufs=2)
                        for blk in range(4):
                            b0 = blk * 32
                            srcb = b0 + (32 if blk % 2 == 0 else -32)
                            nc.gpsimd.dma_start(
                                out=rot[b0 : b0 + 32, :], in_=raw[srcb : srcb + 32, :]
                            )
                        t1 = rope_sc.tile([P, S], BF16, tag="t1", name="rope_t1", bufs=2)
                        t2 = rope_sc.tile([P, S], BF16, tag="t2", name="rope_t2", bufs=2)
                        nc.vector.tensor_mul(t1[:], raw[:], cos2q[:] if isq else cos2k[:])
                        nc.vector.tensor_mul(t2[:], rot[:], sin2q[:] if isq else sin2k[:])
                        nc.vector.tensor_add(
                            (qhat if isq else khat)[:, mk, :], t1[:], t2[:]
                        )
                        if isq:
                            # inverse-rms of this q pair, folded into qhat now
                            nc.scalar.activation(
                                qt[:], qt[:], AF.Sqrt, bias=eps1[0:2, :],
                                scale=1.0 / HD,
                            )
                            nc.vector.reciprocal_approx_fast(qt[:], qt[:])
                            qbf = qkn.tile([2, S], BF16, tag="qbf", name="qbf", bufs=2)
                            nc.vector.tensor_copy(qbf[:], qt[:])
                            for sch in range(2):
                                pe = psum.tile([P, 512], F32, tag="ps", name="ps_erq")
                                nc.tensor.matmul(
                                    pe[:], E2[:],
                                    qbf[:, sch * 512 : (sch + 1) * 512],
                                    start=True, stop=True,
                                )
                                nc.vector.tensor_mul(
                                    qhat[:, mk, sch * 512 : (sch + 1) * 512],
                                    qhat[:, mk, sch * 512 : (sch + 1) * 512], pe[:],
                                )

                    # q inverse-rms per m-tile pair
                    kssT = qkn.tile([P, NT, H], F32, name="kssT", padded_shape=[P, NT, H + 1])
                    for kt in range(NT):
                        nc.sync.dma_start(
                            out=kssT[:, kt, :],
                            in_=kss_dram.ap()[:, kt * P : (kt + 1) * P].rearrange(
                                "h p -> p h"
                            ),
                        )
                    for kt in range(NT):
                        nc.scalar.activation(
                            kssT[:, kt, :], kssT[:, kt, :], AF.Sqrt,
                            bias=epsk[:], scale=1.0,
                        )
                        nc.vector.reciprocal_approx_fast(
                            invrk8[:, kt, :], kssT[:, kt, :]
                        )

                    # q inverse-rms handled inline above

                    # v
                    with tc.tile_pool(name="wv_pool", bufs=3) as wv_pool:
                        for nch in range(4):
                            c0 = nch * 260
                            wt = wv_pool.tile([P, NT, 260], BF16, tag="wv", name="wv_t")
                            nc.sync.dma_start(
                                out=wt[:, 0:4, :], in_=d["wv4"][nch, :, 0:4, :]
                            )
                            nc.sync.dma_start(
                                out=wt[:, 4:8, :], in_=d["wv4"][nch, :, 4:8, :]
                            )
                            for st in range(NT):
                                ps = psum.tile([P, 260], F32, tag="ps", name="ps_v")
                                for kt in range(NT):
                                    nc.tensor.matmul(
                                        ps[:], hT[:, kt, st * P : (st + 1) * P],
                                        wt[:, kt, :],
                                        start=(kt == 0), stop=False,
                                    )
                                nc.tensor.matmul(
                                    ps[:], ones1[:], bv[:, c0 : c0 + 260],
                                    start=False, stop=True,
                                )
                                nc.vector.tensor_copy(
                                    v_sb[:, st, c0 : c0 + 260], ps[:]
                                )

                # ---- Phase D: attention ----
                with tc.tile_pool(name="ppool", bufs=3) as ppool, tc.tile_pool(
                    name="avp", bufs=2, space="PSUM"
                ) as avp, tc.tile_pool(name="att_sc", bufs=2) as att_sc:

                    def qk_exp(h, qch):
                        mk, hh = h // 2, h % 2
                        rb = 64 * hh
                        pT = ppool.tile([P, NT, 512], BF16, tag="pT", name="pT")
                        for kt in range(NT):
                            ps_s = psum.tile([P, 512], F32, tag="ps", name="ps_s")
                            nc.tensor.matmul(
                                ps_s[:],
                                khat[rb : rb + 64, mk, kt * P : (kt + 1) * P],
                                qhat[rb : rb + 64, mk, qch * 512 : (qch + 1) * 512],
                                start=True, stop=True,
                            )
                            nc.scalar.activation(
                                pT[:, kt, :], ps_s[:], AF.Exp,
                                scale=invrk8[:, kt, h : h + 1],
                            )
                        return pT

                    def av_div(h, qch, pT):
                        mk, hh = h // 2, h % 2
                        rb = 64 * hh
                        ps_av = avp.tile([65, 512], F32, tag="ps_av", name="ps_av")
                        for kt in range(NT):
                            nc.tensor.matmul(
                                ps_av[:], v_sb[:, kt, h * 65 : h * 65 + 65],
                                pT[:, kt, :],
                                start=(kt == 0), stop=(kt == NT - 1),
                            )
                        o65 = att_sc.tile([65, 512], F32, tag="o65", name="o65")
                        nc.vector.tensor_copy(o65[:], ps_av[:])
                        o65b = att_sc.tile([65, 512], BF16, tag="o65b", name="o65b")
                        nc.vector.tensor_copy(o65b[:], o65[:])
                        pb = psum.tile([64, 512], F32, tag="ps", name="ps_bc")
                        nc.tensor.matmul(pb[:], e65[:], o65b[:], start=True, stop=True)
                        rb64 = att_sc.tile([64, 512], F32, tag="rb64", name="rb64")
                        nc.vector.reciprocal_approx_fast(rb64[:], pb[:])
                        ob = att_sc.tile([64, 512], BF16, tag="ob", name="ob")
                        nc.vector.tensor_mul(ob[:], o65[0:64, :], rb64[:])
                        nc.sync.dma_start(
                            out=ohat[rb : rb + 64, mk, qch * 512 : (qch + 1) * 512],
                            in_=ob[:],
                        )

                    prev = None
                    for h in range(H):
                        for qch in range(2):
                            pT = qk_exp(h, qch)
                            if prev is not None:
                                av_div(*prev)
                            prev = (h, qch, pT)
                    av_div(*prev)

            # ---- Phase E: proj + residual 1 (in place on xT) ----
            with tc.tile_pool(name="wproj_pool", bufs=3) as wproj_pool:
                for dt in range(NT):
                    wt = wproj_pool.tile([P, NT, P], BF16, tag="wproj", name="wproj_t")
                    nc.sync.dma_start(out=wt[:, 0:4, :], in_=d["wproj8"][dt, :, 0:4, :])
                    nc.sync.dma_start(out=wt[:, 4:8, :], in_=d["wproj8"][dt, :, 4:8, :])
                    for qch in range(2):
                        ps = psum.tile([P, 512], F32, tag="ps", name="ps_proj")
                        for kt in range(NT):
                            nc.tensor.matmul(
                                ps[:], wt[:, kt, :],
                                ohat[:, kt, qch * 512 : (qch + 1) * 512],
                                start=(kt == 0), stop=(kt == NT - 1),
                            )
                        nc.vector.affine_then_add(
                            xT[:, dt, qch * 512 : (qch + 1) * 512],
                            ps[:], xT[:, dt, qch * 512 : (qch + 1) * 512],
                            scale=g1[:, dt : dt + 1], bias=g1b[:, dt : dt + 1],
                        )

        # ======= Phases F-H =======
        with ExitStack() as mctx:
            mlp = mctx.enter_context(tc.tile_pool(name="mlp", bufs=1))

            rms_invb(xT)

            h2T = mlp.tile([P, NT, S], BF16, tag="h2T", name="h2T")
            modulate(xT, h2T, a2, sh2)

            gg = mlp.tile([P, NKT12, S], BF16, tag="gg", name="gg")
            with tc.tile_pool(name="w12_pool", bufs=4) as w12_pool, tc.tile_pool(
                name="mlp_sc", bufs=2
            ) as mlp_sc:
                for j in range(NKT12):
                    outs = []
                    for part in range(2):
                        m = j + part * NKT12
                        wt = w12_pool.tile([P, NT, P], BF16, tag="w12", name="w12_t")
                        nc.sync.dma_start(out=wt[:, 0:4, :], in_=d["w1244"][m, :, 0:4, :])
                        nc.sync.dma_start(out=wt[:, 4:8, :], in_=d["w1244"][m, :, 4:8, :])
                        o = mlp_sc.tile([P, S], BF16, tag=f"mlp{part}", name=f"mlp{part}")
                        for sch in range(2):
                            ps = psum.tile([P, 512], F32, tag="ps", name="ps_mlp")
                            for kt in range(NT):
                                nc.tensor.matmul(
                                    ps[:], wt[:, kt, :],
                                    h2T[:, kt, sch * 512 : (sch + 1) * 512],
                                    start=(kt == 0), stop=(kt == NT - 1),
                                )
                            nc.scalar.activation(
                                o[:, sch * 512 : (sch + 1) * 512], ps[:],
                                AF.Silu if part == 0 else AF.Identity,
                                bias=b12T[:, m : m + 1],
                            )
                        outs.append(o)
                    nc.vector.tensor_mul(gg[:, j, :], outs[0][:], outs[1][:])

            # w3 + residual 2 (in place on xT)
            with tc.tile_pool(name="w3_pool", bufs=3) as w3_pool:
                for dt in range(NT):
                    wt = w3_pool.tile([P, NKT12, P], BF16, tag="w3", name="w3_t")
                    for k0, k1 in ((0, 6), (6, 11), (11, 17), (17, NKT12)):
                        nc.sync.dma_start(
                            out=wt[:, k0:k1, :], in_=d["w38"][dt, :, k0:k1, :]
                        )
                    for qch in range(2):
                        ps = psum.tile([P, 512], F32, tag="ps", name="ps_w3")
                        for kt in range(NKT12):
                            nc.tensor.matmul(
                                ps[:], wt[:, kt, :],
                                gg[:, kt, qch * 512 : (qch + 1) * 512],
                                start=(kt == 0), stop=(kt == NKT12 - 1),
                            )
                        nc.vector.affine_then_add(
                            xT[:, dt, qch * 512 : (qch + 1) * 512],
                            ps[:], xT[:, dt, qch * 512 : (qch + 1) * 512],
                            scale=g2[:, dt : dt + 1], bias=g2b3[:, dt : dt + 1],
                        )

            # ---- Phase H ----
            with tc.tile_pool(name="yout", bufs=3) as ypool:
                for st in range(NT):
                    y = ypool.tile([P, D], F32, tag="y", name="y")
                    for g4 in range(2):
                        pt = psum.tile([P, 512], F32, tag="ps", name="ps_tr2")
                        for j in range(4):
                            dt = g4 * 4 + j
                            nc.tensor.transpose(
                                pt[:, j * P : (j + 1) * P],
                                xT[:, dt, st * P : (st + 1) * P],
                                ident[:],
                            )
                        for j in range(4):
                            dt = g4 * 4 + j
                            nc.scalar.activation(
                                y[:, dt * P : (dt + 1) * P],
                                pt[:, j * P : (j + 1) * P],
                                AF.Copy,
                            )
                    nc.sync.dma_start(
                        out=d["out"][st * P : (st + 1) * P, 0:512], in_=y[:, 0:512]
                    )
                    nc.sync.dma_start(
                        out=d["out"][st * P : (st + 1) * P, 512:D], in_=y[:, 512:D]
                    )


def kernel(**inputs):
    inputs = {k: np.asarray(v) for k, v in inputs.items()}
    if "nc" not in _CACHE:
        _CACHE["nc"] = build_bass()
    nc = _CACHE["nc"]

    consts = _prep_weights(inputs)
    base = {}
    for k, v in consts.items():
        if k in BF16_NAMES:
            base[k] = np.ascontiguousarray(v).astype(ml_dtypes.bfloat16)
        else:
            base[k] = np.ascontiguousarray(v).astype(np.float32)

    in_maps = []
    for core in range(B):
        m = dict(base)
        m["x"] = np.ascontiguousarray(inputs["x"][core]).astype(np.float32)
        m["cT"] = _to_pmaj(inputs["c"][core]).astype(np.float32)
        in_maps.append(m)

    res = run_bass_kernel_spmd(
        nc, in_maps, core_ids=list(range(B)), **_CACHE.get("run_kwargs", {})
    )
    _CACHE["last_results"] = res
    return np.stack([res.results[i]["out"] for i in range(B)], axis=0)


if __name__ == "__main__":
    build_bass()
    print("built ok")



# revision 38
# speedup vs baseline: 1.5012x; 1.4718x over previous
"""JiT/DiT transformer block (adaLN + attention + SwiGLU) on 8 TRN2 NeuronCores.

Data-parallel over batch: core i computes batch element i end-to-end; no
collectives. Activations are kept "transposed" on device ([channel, seq]) so
per-channel modulation/bias are per-partition scalars; attention scores are
produced directly in [k, q] layout (softmax denominator via a ones-row
appended to V inside the AV matmul). The big GEMMs (qkv/v/proj/AV/w12/w3) run
fp8-e4m3 DoubleRow (weights pre-scaled x1024, descale folded into the
post-matmul op); scores stay bf16; the residual stream stays fp32. Weights are
host-prepacked into [m, p, kt, c] so every DMA is contiguous; the adaLN GEMV
is split so only its first half gates the attention branch.
"""

import sys

sys.path.insert(0, "/opt/trn_rl_repo")

import numpy as np
import ml_dtypes

import concourse.bacc as bacc
import concourse.bass as bass
import concourse.mybir as mybir
from concourse.tile import TileContext
from concourse.bass_utils import run_bass_kernel_spmd

F32 = mybir.dt.float32
BF16 = mybir.dt.bfloat16
F8 = mybir.dt.float8e4
AF = mybir.ActivationFunctionType
ALU = mybir.AluOpType
DR = mybir.MatmulPerfMode.DoubleRow

B, S, D, H = 8, 1024, 1024, 16
HD = D // H  # 64
INNER = 2730
INNER_P = 2816  # 22*128
P = 128
NT = 8
NKT12 = INNER_P // P  # 22
EPS = 1e-6
WS = 1024.0  # fp8 weight pre-scale
GS = 16.0  # gg (gated) pre-scale
EXPB = -2.5  # softmax exp bias (cancels in the division)

_CACHE = {}


def _to_pmaj(v):
    return np.ascontiguousarray(v.reshape(-1, P).T)


def _rope_perm():
    ev = np.arange(0, HD, 2)
    od = np.arange(1, HD, 2)
    perm = np.concatenate([ev, od])
    partner = np.concatenate([od, ev])
    return perm, partner


def _prep_weights(inp):
    """Host-side layout/dtype prep (reordering/padding/scaling only)."""
    perm, partner = _rope_perm()
    chperm = (np.arange(D).reshape(H, HD)[:, perm]).reshape(-1)

    w_qkv, b_qkv = inp["w_qkv"], inp["b_qkv"]
    wq = w_qkv[:, 0:D][:, chperm]
    wk = w_qkv[:, D : 2 * D][:, chperm]
    wv = w_qkv[:, 2 * D :]
    bq = b_qkv[0:D][chperm]
    bk = b_qkv[D : 2 * D][chperm]
    bv = b_qkv[2 * D :]
    wv_ext = np.zeros((D, H * 65), np.float32)
    bv_ext = np.zeros((H * 65,), np.float32)
    for h in range(H):
        wv_ext[:, h * 65 : h * 65 + 64] = wv[:, h * 64 : (h + 1) * 64]
        bv_ext[h * 65 : h * 65 + 64] = bv[h * 64 : (h + 1) * 64]
        bv_ext[h * 65 + 64] = 1.0
    wqk_cat = np.concatenate([wq, wk], axis=1)  # [D, 2048]

    w12, b12 = inp["w12"], inp["b12"]
    w12p = np.zeros((D, 2 * INNER_P), np.float32)
    b12p = np.zeros((2 * INNER_P,), np.float32)
    w12p[:, :INNER] = w12[:, :INNER]
    w12p[:, INNER_P : INNER_P + INNER] = w12[:, INNER:]
    b12p[:INNER] = b12[:INNER]
    b12p[INNER_P : INNER_P + INNER] = b12[INNER:]
    w3p = np.zeros((INNER_P, D), np.float32)
    w3p[:INNER] = inp["w3"]

    # rope tiles [128, S]: two stacked 64-row head-local blocks
    sign = np.where(np.arange(HD) < HD // 2, -1.0, 1.0).astype(np.float32)
    cos, sin = inp["rope_cos"], inp["rope_sin"]

    def rope_tiles(scale_vec):
        c64 = cos[:, perm].T * scale_vec[perm][:, None]
        s64 = (sin[:, perm].T * sign[:, None]) * scale_vec[partner][:, None]
        return (
            np.concatenate([c64, c64], 0).astype(np.float32),
            np.concatenate([s64, s64], 0).astype(np.float32),
        )

    cq, sq = rope_tiles(inp["qn_scale"])
    ck, sk = rope_tiles(inp["kn_scale"])

    E2 = np.zeros((2, P), np.float32)
    E2[0, 0:64] = 1.0
    E2[1, 64:128] = 1.0
    e65 = np.zeros((65, 64), np.float32)
    e65[64, :] = 1.0
    bo2 = np.zeros((P, 2), np.float32)
    bo2[0:64, 0] = 1.0
    bo2[64:128, 1] = 1.0

    bqk_T = np.stack(
        [bq.reshape(NT, P)[m] for m in range(NT)]
        + [bk.reshape(NT, P)[m] for m in range(NT)],
        axis=1,
    )

    b12T = _to_pmaj(b12p)
    b12T[:, NKT12:] *= GS  # part-1 bias pre-scaled with gg

    # [m, p, kt, c] prepack: weight W[K, M] consumed as m-tiles of [P, kt, C]
    def pack(w, C):
        K, M = w.shape
        return np.ascontiguousarray(
            w.reshape(K // P, P, M // C, C).transpose(2, 1, 0, 3)
        )

    return {
        "wqkv16": pack(wqk_cat * WS, P),
        "wv1": pack(wv_ext * WS, H * 65)[0],
        "wproj8": pack(inp["w_proj"] * WS, P),
        "w1244": pack(w12p * WS, P),
        "w38": pack(w3p * WS, P),
        "wada12": pack(inp["w_ada"], 512),
        "bqk_T": bqk_T, "bv_ext": (bv_ext * WS)[None, :],
        "b12T": b12T, "bprojT": _to_pmaj(inp["b_proj"]),
        "b3T": _to_pmaj(inp["b3"]), "n1T": _to_pmaj(inp["norm1_scale"]),
        "n2T": _to_pmaj(inp["norm2_scale"]), "b_ada": inp["b_ada"][None, :],
        "E2": E2, "e65": e65, "bo2": bo2, "ones1": np.ones((1, P), np.float32),
        "ident": np.eye(P, dtype=np.float32),
        "cos2q": cq, "sin2q": sq, "cos2k": ck, "sin2k": sk,
    }


BF16_NAMES = {
    "wada12", "bv_ext", "E2", "e65", "bo2", "ones1",
    "cos2q", "sin2q", "cos2k", "sin2k",
}
F8_NAMES = {"wqkv16", "wv1", "wproj8", "w1244", "w38"}


def build_bass():
    nc = bacc.Bacc("TRN2", target_bir_lowering=False, debug=False, num_devices=8)

    def par(name, shape, dt, out=False):
        return nc.declare_dram_parameter(name, list(shape), dt, isOutput=out)

    d = {
        "x": par("x", [S, D], F32),
        "cT": par("cT", [P, NT], F32),
        "wqkv16": par("wqkv16", [16, P, NT, P], F8),
        "wv1": par("wv1", [P, NT, H * 65], F8),
        "wproj8": par("wproj8", [NT, P, NT, P], F8),
        "w1244": par("w1244", [2 * NKT12, P, NT, P], F8),
        "w38": par("w38", [NT, P, NKT12, P], F8),
        "wada12": par("wada12", [12, P, NT, 512], BF16),
        "bqk_T": par("bqk_T", [P, 16], F32),
        "bv_ext": par("bv_ext", [1, H * 65], BF16),
        "b12T": par("b12T", [P, 2 * NKT12], F32),
        "bprojT": par("bprojT", [P, NT], F32),
        "b3T": par("b3T", [P, NT], F32),
        "n1T": par("n1T", [P, NT], F32),
        "n2T": par("n2T", [P, NT], F32),
        "b_ada": par("b_ada", [1, 6 * D], F32),
        "E2": par("E2", [2, P], BF16),
        "e65": par("e65", [65, 64], BF16),
        "bo2": par("bo2", [P, 2], BF16),
        "ones1": par("ones1", [1, P], BF16),
        "ident": par("ident", [P, P], F32),
        "cos2q": par("cos2q", [P, S], BF16),
        "sin2q": par("sin2q", [P, S], BF16),
        "cos2k": par("cos2k", [P, S], BF16),
        "sin2k": par("sin2k", [P, S], BF16),
        "out": par("out", [S, D], F32, out=True),
    }
    mods_dram = nc.dram_tensor("mods_scratch", [1, 6 * D], F32)
    kss_dram = nc.dram_tensor("kss_scratch", [H, S], BF16)

    with TileContext(nc) as tc:
        _body(nc, tc, d, mods_dram, kss_dram)
    nc.compile()
    return nc


def _body(nc, tc, d, mods_dram, kss_dram):
    from contextlib import ExitStack

    with ExitStack() as ctx:
        const = ctx.enter_context(tc.tile_pool(name="const", bufs=1))
        persist = ctx.enter_context(tc.tile_pool(name="persist", bufs=1))
        small = ctx.enter_context(tc.tile_pool(name="small", bufs=1))
        scratch = ctx.enter_context(tc.tile_pool(name="scratch", bufs=2))
        psum = ctx.enter_context(tc.tile_pool(name="psum", bufs=6, space="PSUM"))
        wada_pool = ctx.enter_context(tc.tile_pool(name="wada_pool", bufs=3))
        ada_sc = ctx.enter_context(tc.tile_pool(name="ada_sc", bufs=2))

        def load_const(key, shape, dt, pool=None):
            t = (pool or const).tile(list(shape), dt, tag=key, name=key + "_sb")
            nc.sync.dma_start(out=t[:], in_=d[key][:])
            return t

        cT = load_const("cT", [P, NT], F32)
        bqkT = load_const("bqk_T", [P, 16], F32)
        bv = load_const("bv_ext", [1, H * 65], BF16)
        b12T = load_const("b12T", [P, 2 * NKT12], F32)
        bprojT = load_const("bprojT", [P, NT], F32)
        b3T = load_const("b3T", [P, NT], F32)
        n1T = load_const("n1T", [P, NT], F32)
        n2T = load_const("n2T", [P, NT], F32)
        bo2 = load_const("bo2", [P, 2], BF16)
        e65 = load_const("e65", [65, 64], BF16)
        ones1 = load_const("ones1", [1, P], BF16)
        ident = load_const("ident", [P, P], F32)
        ones128 = const.tile([P, P], BF16, tag="ones128", name="ones128")
        nc.vector.memset(ones128[:], 1.0)
        eps1 = const.tile([P, 1], F32, tag="eps1", name="eps1")
        nc.vector.memset(eps1[:], EPS)
        epsk = const.tile([P, 1], F32, tag="epsk", name="epsk")
        nc.vector.memset(epsk[:], HD * EPS)
        expb = const.tile([P, 1], F32, tag="expb", name="expb")
        nc.vector.memset(expb[:], EXPB)

        # residual stream lives here, updated in place
        xT = persist.tile([P, NT, S], F32, tag="bigf32", name="xT")
        invb = persist.tile([P, S], F32, tag="invb", name="invb")
        invrk8 = small.tile([P, NT, H], F32, name="invrk8", padded_shape=[P, NT, H + 1])
        invrk8b = small.tile(
            [P, NT, H], BF16, name="invrk8b", padded_shape=[P, NT, H + 1]
        )
        modsT = small.tile([P, 48], F32, name="modsT")

        def rms_invb(zT):
            # invb[:, ch*512:...] = 1/sqrt(mean_d z^2 + eps) (rows identical)
            for ch in range(2):
                ms = None
                for dt in range(NT):
                    sq = scratch.tile([P, 512], BF16, tag="sqd", name="sqd")
                    nc.vector.tensor_mul(
                        sq[:],
                        zT[:, dt, ch * 512 : (ch + 1) * 512],
                        zT[:, dt, ch * 512 : (ch + 1) * 512],
                    )
                    if dt == 0:
                        ms = psum.tile([P, 512], F32, tag="ps", name="ps_ms")
                    nc.tensor.matmul(
                        ms[:], ones128[:], sq[:],
                        start=(dt == 0), stop=(dt == NT - 1),
                    )
                rms = scratch.tile([P, 512], F32, tag="rms", name="rms")
                nc.scalar.activation(rms[:], ms[:], AF.Sqrt, bias=eps1[:], scale=1.0 / D)
                nc.vector.reciprocal_approx_fast(
                    invb[:, ch * 512 : (ch + 1) * 512], rms[:]
                )

        def modulate(zT, dstT, aa, sh):
            for dt in range(NT):
                tmp = scratch.tile([P, S], F32, tag="htmp", name="htmp")
                nc.vector.tensor_mul(tmp[:], zT[:, dt, :], invb[:])
                nc.vector.tensor_scalar(
                    dstT[:, dt, :], tmp[:], aa[:, dt : dt + 1], sh[:, dt : dt + 1],
                    op0=ALU.mult, op1=ALU.add,
                )

        # ---- adaLN GEMV pieces (split so only n=0..5 gate attention) ----
        cT_silu = small.tile([P, NT], F32, name="cT_silu")
        nc.scalar.activation(cT_silu[:], cT[:], AF.Silu)
        cT_bf = small.tile([P, NT], BF16, name="cT_bf")
        nc.vector.tensor_copy(cT_bf[:], cT_silu[:])

        def ada_chunk(n):
            ps = psum.tile([1, 512], F32, tag="ps", name="ps_ada")
            wt = wada_pool.tile([P, NT, 512], BF16, tag="wada", name="wada_t")
            nc.sync.dma_start(out=wt[:, 0:4, :], in_=d["wada12"][n, :, 0:4, :])
            nc.sync.dma_start(out=wt[:, 4:8, :], in_=d["wada12"][n, :, 4:8, :])
            for kt in range(NT):
                nc.tensor.matmul(
                    ps[:], cT_bf[:, kt : kt + 1], wt[:, kt, :],
                    start=(kt == 0), stop=(kt == NT - 1),
                )
            bch = ada_sc.tile([1, 512], F32, tag="bch", name="bada_ch")
            nc.sync.dma_start(out=bch[:], in_=d["b_ada"][:, n * 512 : (n + 1) * 512])
            mch = ada_sc.tile([1, 512], F32, tag="mch", name="mods_ch")
            nc.vector.tensor_add(mch[:], ps[:], bch[:])
            nc.sync.dma_start(out=mods_dram[:, n * 512 : (n + 1) * 512], in_=mch[:])

        def mods_load(c0, c1):
            nc.sync.dma_start(
                out=modsT[:, c0:c1],
                in_=mods_dram.ap()[0, c0 * P : c1 * P].rearrange("(t p) -> p t", p=P),
            )

        # ======= Phases B-E =======
        with ExitStack() as actx:
            ho = actx.enter_context(tc.tile_pool(name="ho", bufs=1))
            hT = ho.tile([P, NT, S], F8, tag="hT", name="hT")
            ohat = ho.tile([P, NT, S], F8, tag="ohat", name="ohat")

            # ---- Phase B (+ ada n=0..5 interleaved) ----
            with tc.tile_pool(name="xin_pool", bufs=3) as xin_pool:
                for st in range(NT):
                    xin = xin_pool.tile([P, D], F32, tag="xin", name="xin")
                    nc.sync.dma_start(
                        out=xin[:, 0:512], in_=d["x"][st * P : (st + 1) * P, 0:512]
                    )
                    nc.sync.dma_start(
                        out=xin[:, 512:D], in_=d["x"][st * P : (st + 1) * P, 512:D]
                    )
                    for g4 in range(2):
                        pt = psum.tile([P, 512], F32, tag="ps", name="ps_tr")
                        for j in range(4):
                            dt = g4 * 4 + j
                            nc.tensor.transpose(
                                pt[:, j * P : (j + 1) * P],
                                xin[:, dt * P : (dt + 1) * P],
                                ident[:],
                            )
                        for j in range(4):
                            dt = g4 * 4 + j
                            nc.scalar.activation(
                                xT[:, dt, st * P : (st + 1) * P],
                                pt[:, j * P : (j + 1) * P],
                                AF.Copy,
                            )
                    if st < 6:
                        ada_chunk(st)

            rms_invb(xT)

            mods_load(0, 24)
            a1 = small.tile([P, NT], F32, name="a1")
            nc.vector.tensor_scalar_add(a1[:], modsT[:, 8:16], 1.0)
            nc.vector.tensor_mul(a1[:], a1[:], n1T[:])
            sh1 = modsT[:, 0:8]
            g1 = modsT[:, 16:24]
            g1b = small.tile([P, NT], F32, name="g1b")
            nc.vector.tensor_mul(g1b[:], g1, bprojT[:])
            g1s = small.tile([P, NT], F32, name="g1s")
            nc.vector.tensor_scalar_mul(g1s[:], g1, 1.0 / WS)

            modulate(xT, hT, a1, sh1)

            # ---- Phases C + D in a scoped block ----
            with ExitStack() as cctx:
                qk = cctx.enter_context(tc.tile_pool(name="qk", bufs=1))
                qhat = qk.tile([P, NT, S], BF16, tag="qhat", name="qhat")
                khat = qk.tile([P, NT, S], BF16, tag="khat", name="khat")
                v_sb = qk.tile([P, NT, H * 65], F8, tag="v", name="v_sb")

                with ExitStack() as qctx:
                    ropec = qctx.enter_context(tc.tile_pool(name="ropec", bufs=1))
                    qkn = qctx.enter_context(tc.tile_pool(name="qkn", bufs=1))
                    wqk_pool = qctx.enter_context(tc.tile_pool(name="wqk_pool", bufs=4))
                    rope_sc = qctx.enter_context(tc.tile_pool(name="rope_sc", bufs=2))

                    cos2q = load_const("cos2q", [P, S], BF16, pool=ropec)
                    sin2q = load_const("sin2q", [P, S], BF16, pool=ropec)
                    cos2k = load_const("cos2k", [P, S], BF16, pool=ropec)
                    sin2k = load_const("sin2k", [P, S], BF16, pool=ropec)
                    E2 = load_const("E2", [2, P], BF16, pool=ropec)

                    def emit_ss(m, isq, mk, sqs_pair):
                        # sum-of-squares -> 1/rms (with the 1/8 attn scale for k);
                        # for q, immediately broadcast and fold into qhat.
                        kstage = None
                        for sch in range(2):
                            ss = psum.tile([2, 512], F32, tag="ps", name="ps_ss")
                            nc.tensor.matmul(
                                ss[:], bo2[:], sqs_pair[sch][:], start=True, stop=True
                            )
                            sr = scratch.tile([2, 512], F32, tag="sr", name="sr", bufs=4)
                            if isq:
                                nc.scalar.activation(
                                    sr[:], ss[:], AF.Sqrt,
                                    bias=eps1[0:2, :], scale=1.0 / HD,
                                )
                                nc.vector.reciprocal_approx_fast(sr[:], sr[:])
                                srb = scratch.tile(
                                    [2, 512], BF16, tag="srb", name="srb", bufs=4
                                )
                                nc.vector.tensor_copy(srb[:], sr[:])
                                pe = psum.tile([P, 512], F32, tag="ps", name="ps_erq")
                                nc.tensor.matmul(
                                    pe[:], E2[:], srb[:], start=True, stop=True
                                )
                                nc.vector.tensor_mul(
                                    qhat[:, mk, sch * 512 : (sch + 1) * 512],
                                    qhat[:, mk, sch * 512 : (sch + 1) * 512], pe[:],
                                )
                            else:
                                if kstage is None:
                                    kstage = qkn.tile(
                                        [2, S], BF16, tag="kstage", name="kstage", bufs=2
                                    )
                                nc.scalar.activation(
                                    sr[:], ss[:], AF.Sqrt,
                                    bias=epsk[0:2, :], scale=1.0,
                                )
                                nc.vector.reciprocal_approx_fast(sr[:], sr[:])
                                nc.vector.tensor_copy(
                                    kstage[:, sch * 512 : (sch + 1) * 512], sr[:]
                                )
                                nc.sync.dma_start(
                                    out=kss_dram[
                                        2 * mk : 2 * mk + 2, sch * 512 : (sch + 1) * 512
                                    ],
                                    in_=kstage[:, sch * 512 : (sch + 1) * 512],
                                )

                    pend = []
                    for m in range(16):
                        isq = m < NT
                        mk = m if isq else m - NT
                        wt = wqk_pool.tile([P, NT, P], F8, tag="wqk", name="wqk_t")
                        nc.sync.dma_start(out=wt[:, 0:4, :], in_=d["wqkv16"][m, :, 0:4, :])
                        nc.sync.dma_start(out=wt[:, 4:8, :], in_=d["wqkv16"][m, :, 4:8, :])
                        raw = rope_sc.tile([P, S], BF16, tag="raw", name="qk_raw")
                        sqs_pair = []
                        for sch in range(2):
                            ps = psum.tile([P, 512], F32, tag="ps", name="ps_qkv")
                            for kp in range(4):
                                nc.tensor.matmul(
                                    ps[:], wt[:, 2 * kp : 2 * kp + 2, :],
                                    hT[:, 2 * kp : 2 * kp + 2, sch * 512 : (sch + 1) * 512],
                                    start=(kp == 0), stop=(kp == 3), perf_mode=DR,
                                )
                            nc.vector.tensor_scalar(
                                raw[:, sch * 512 : (sch + 1) * 512], ps[:],
                                1.0 / WS, bqkT[:, m : m + 1],
                                op0=ALU.mult, op1=ALU.add,
                            )
                            sqs = scratch.tile(
                                [P, 512], BF16, tag="sqs", name="sqs", bufs=6
                            )
                            nc.vector.tensor_mul(
                                sqs[:],
                                raw[:, sch * 512 : (sch + 1) * 512],
                                raw[:, sch * 512 : (sch + 1) * 512],
                            )
                            sqs_pair.append(sqs)
                        # software-pipelined (depth 2): older sum-of-squares mms
                        if len(pend) == 2:
                            emit_ss(*pend.pop(0))
                        pend.append((m, isq, mk, sqs_pair))

                        rot = rope_sc.tile([P, S], BF16, tag="rot", name="rot", bufs=2)
                        for blk in range(4):
                            b0 = blk * 32
                            srcb = b0 + (32 if blk % 2 == 0 else -32)
                            nc.gpsimd.dma_start(
                                out=rot[b0 : b0 + 32, :], in_=raw[srcb : srcb + 32, :]
                            )
                        t1 = rope_sc.tile([P, S], BF16, tag="t1", name="rope_t1", bufs=2)
                        t2 = rope_sc.tile([P, S], BF16, tag="t2", name="rope_t2", bufs=2)
                        nc.vector.tensor_mul(t1[:], raw[:], cos2q[:] if isq else cos2k[:])
                        nc.vector.tensor_mul(t2[:], rot[:], sin2q[:] if isq else sin2k[:])
                        nc.vector.tensor_add(
                            (qhat if isq else khat)[:, mk, :], t1[:], t2[:]
                        )
                    for pp in pend:
                        emit_ss(*pp)

                    # k inverse-rms reload (DRAM transpose roundtrip; already 1/(8*rms))
                    for kt in range(NT):
                        nc.sync.dma_start(
                            out=invrk8b[:, kt, :],
                            in_=kss_dram.ap()[:, kt * P : (kt + 1) * P].rearrange(
                                "h p -> p h"
                            ),
                        )
                    nc.vector.tensor_copy(invrk8[:], invrk8b[:])

                    # v (single fp8 weight tile, stationary = activations)
                    v_w = qk.tile([P, NT, H * 65], F8, tag="v_w", name="v_w")
                    for kb in range(4):
                        nc.sync.dma_start(
                            out=v_w[:, 2 * kb : 2 * kb + 2, :],
                            in_=d["wv1"][:, 2 * kb : 2 * kb + 2, :],
                        )
                    for st in range(NT):
                        for nch in range(4):
                            c0 = nch * 260
                            ps = psum.tile([P, 260], F32, tag="ps", name="ps_v")
                            for kp in range(4):
                                nc.tensor.matmul(
                                    ps[:],
                                    hT[:, 2 * kp : 2 * kp + 2, st * P : (st + 1) * P],
                                    v_w[:, 2 * kp : 2 * kp + 2, c0 : c0 + 260],
                                    start=(kp == 0), stop=False, perf_mode=DR,
                                )
                            nc.tensor.matmul(
                                ps[:], ones1[:], bv[:, c0 : c0 + 260],
                                start=False, stop=True, skip_group_check=True,
                            )
                            nc.vector.tensor_scalar_mul(
                                v_sb[:, st, c0 : c0 + 260], ps[:], 1.0 / WS
                            )

                # ---- Phase D: attention ----
                with tc.tile_pool(name="ppool", bufs=3) as ppool, tc.tile_pool(
                    name="avp", bufs=2, space="PSUM"
                ) as avp, tc.tile_pool(name="att_sc", bufs=2) as att_sc:

                    def qk_exp(h, qch):
                        mk, hh = h // 2, h % 2
                        rb = 64 * hh
                        pT = ppool.tile([P, NT, 512], F8, tag="pT", name="pT")
                        for kt in range(NT):
                            ps_s = psum.tile([P, 512], F32, tag="ps", name="ps_s")
                            nc.tensor.matmul(
                                ps_s[:],
                                khat[rb : rb + 64, mk, kt * P : (kt + 1) * P],
                                qhat[rb : rb + 64, mk, qch * 512 : (qch + 1) * 512],
                                start=True, stop=True,
                            )
                            nc.scalar.activation(
                                pT[:, kt, :], ps_s[:], AF.Exp,
                                scale=invrk8[:, kt, h : h + 1], bias=expb[:],
                            )
                        return pT

                    def av_div(h, qch, pT):
                        mk, hh = h // 2, h % 2
                        rb = 64 * hh
                        ps_av = avp.tile([65, 512], F32, tag="ps_av", name="ps_av")
                        for kp in range(4):
                            nc.tensor.matmul(
                                ps_av[:],
                                v_sb[:, 2 * kp : 2 * kp + 2, h * 65 : h * 65 + 65],
                                pT[:, 2 * kp : 2 * kp + 2, :],
                                start=(kp == 0), stop=(kp == 3), perf_mode=DR,
                            )
                        o65 = att_sc.tile([65, 512], F32, tag="o65", name="o65")
                        nc.vector.tensor_copy(o65[:], ps_av[:])
                        o65b = att_sc.tile([65, 512], BF16, tag="o65b", name="o65b")
                        nc.vector.tensor_copy(o65b[:], o65[:])
                        pb = psum.tile([64, 512], F32, tag="ps", name="ps_bc")
                        nc.tensor.matmul(pb[:], e65[:], o65b[:], start=True, stop=True)
                        rb64 = att_sc.tile([64, 512], F32, tag="rb64", name="rb64")
                        nc.vector.reciprocal_approx_fast(rb64[:], pb[:])
                        ob = att_sc.tile([64, 512], F8, tag="ob", name="ob")
                        nc.vector.tensor_mul(ob[:], o65[0:64, :], rb64[:])
                        nc.sync.dma_start(
                            out=ohat[rb : rb + 64, mk, qch * 512 : (qch + 1) * 512],
                            in_=ob[:],
                        )

                    prev = None
                    for h in range(H):
                        for qch in range(2):
                            pT = qk_exp(h, qch)
                            if prev is not None:
                                av_div(*prev)
                            prev = (h, qch, pT)
                        if h < 6:
                            # ada n=6..11 stream through the attention phase
                            ada_chunk(6 + h)
                    av_div(*prev)
                    mods_load(24, 48)
                    a2 = small.tile([P, NT], F32, name="a2")
                    nc.vector.tensor_scalar_add(a2[:], modsT[:, 32:40], 1.0)
                    nc.vector.tensor_mul(a2[:], a2[:], n2T[:])
                    sh2 = modsT[:, 24:32]
                    g2 = modsT[:, 40:48]
                    g2b3 = small.tile([P, NT], F32, name="g2b3")
                    nc.vector.tensor_mul(g2b3[:], g2, b3T[:])
                    g2s = small.tile([P, NT], F32, name="g2s")
                    nc.vector.tensor_scalar_mul(g2s[:], g2, 1.0 / (WS * GS))

            # ---- Phase E: proj + residual 1 (in place on xT) ----
            with tc.tile_pool(name="wproj_pool", bufs=3) as wproj_pool:
                for dt in range(NT):
                    wt = wproj_pool.tile([P, NT, P], F8, tag="wproj", name="wproj_t")
                    nc.sync.dma_start(out=wt[:, 0:4, :], in_=d["wproj8"][dt, :, 0:4, :])
                    nc.sync.dma_start(out=wt[:, 4:8, :], in_=d["wproj8"][dt, :, 4:8, :])
                    for qch in range(2):
                        ps = psum.tile([P, 512], F32, tag="ps", name="ps_proj")
                        for kp in range(4):
                            nc.tensor.matmul(
                                ps[:], wt[:, 2 * kp : 2 * kp + 2, :],
                                ohat[:, 2 * kp : 2 * kp + 2, qch * 512 : (qch + 1) * 512],
                                start=(kp == 0), stop=(kp == 3), perf_mode=DR,
                            )
                        nc.vector.affine_then_add(
                            xT[:, dt, qch * 512 : (qch + 1) * 512],
                            ps[:], xT[:, dt, qch * 512 : (qch + 1) * 512],
                            scale=g1s[:, dt : dt + 1], bias=g1b[:, dt : dt + 1],
                        )

        # ======= Phases F-H =======
        with ExitStack() as mctx:
            mlp = mctx.enter_context(tc.tile_pool(name="mlp", bufs=1))

            rms_invb(xT)

            h2T = mlp.tile([P, NT, S], F8, tag="h2T", name="h2T")
            modulate(xT, h2T, a2, sh2)

            gg = mlp.tile([P, NKT12, S], F8, tag="gg", name="gg")
            with tc.tile_pool(name="w12_pool", bufs=4) as w12_pool, tc.tile_pool(
                name="mlp_sc", bufs=2
            ) as mlp_sc:
                for j in range(NKT12):
                    outs = []
                    for part in range(2):
                        m = j + part * NKT12
                        wt = w12_pool.tile([P, NT, P], F8, tag="w12", name="w12_t")
                        nc.sync.dma_start(out=wt[:, 0:4, :], in_=d["w1244"][m, :, 0:4, :])
                        nc.sync.dma_start(out=wt[:, 4:8, :], in_=d["w1244"][m, :, 4:8, :])
                        o = mlp_sc.tile([P, S], BF16, tag=f"mlp{part}", name=f"mlp{part}")
                        for sch in range(2):
                            ps = psum.tile([P, 512], F32, tag="ps", name="ps_mlp")
                            for kp in range(4):
                                nc.tensor.matmul(
                                    ps[:], wt[:, 2 * kp : 2 * kp + 2, :],
                                    h2T[:, 2 * kp : 2 * kp + 2, sch * 512 : (sch + 1) * 512],
                                    start=(kp == 0), stop=(kp == 3), perf_mode=DR,
                                )
                            nc.scalar.activation(
                                o[:, sch * 512 : (sch + 1) * 512], ps[:],
                                AF.Silu if part == 0 else AF.Identity,
                                bias=b12T[:, m : m + 1],
                                scale=(1.0 / WS) if part == 0 else (GS / WS),
                            )
                        outs.append(o)
                    nc.vector.tensor_mul(gg[:, j, :], outs[0][:], outs[1][:])

            # w3 + residual 2 (in place on xT)
            with tc.tile_pool(name="w3_pool", bufs=3) as w3_pool:
                for dt in range(NT):
                    wt = w3_pool.tile([P, NKT12, P], F8, tag="w3", name="w3_t")
                    for k0, k1 in ((0, 6), (6, 12), (12, 18), (18, NKT12)):
                        nc.sync.dma_start(
                            out=wt[:, k0:k1, :], in_=d["w38"][dt, :, k0:k1, :]
                        )
                    for qch in range(2):
                        ps = psum.tile([P, 512], F32, tag="ps", name="ps_w3")
                        for kp in range(11):
                            nc.tensor.matmul(
                                ps[:], wt[:, 2 * kp : 2 * kp + 2, :],
                                gg[:, 2 * kp : 2 * kp + 2, qch * 512 : (qch + 1) * 512],
                                start=(kp == 0), stop=(kp == 10), perf_mode=DR,
                            )
                        nc.vector.affine_then_add(
                            xT[:, dt, qch * 512 : (qch + 1) * 512],
                            ps[:], xT[:, dt, qch * 512 : (qch + 1) * 512],
                            scale=g2s[:, dt : dt + 1], bias=g2b3[:, dt : dt + 1],
                        )

            # ---- Phase H ----
            with tc.tile_pool(name="yout", bufs=3) as ypool:
                for st in range(NT):
                    y = ypool.tile([P, D], F32, tag="y", name="y")
                    for g4 in range(2):
                        pt = psum.tile([P, 512], F32, tag="ps", name="ps_tr2")
                        for j in range(4):
                            dt = g4 * 4 + j
                            nc.tensor.transpose(
                                pt[:, j * P : (j + 1) * P],
                                xT[:, dt, st * P : (st + 1) * P],
                                ident[:],
                            )
                        for j in range(4):
                            dt = g4 * 4 + j
                            nc.scalar.activation(
                                y[:, dt * P : (dt + 1) * P],
                                pt[:, j * P : (j + 1) * P],
                                AF.Copy,
                            )
                    nc.sync.dma_start(
                        out=d["out"][st * P : (st + 1) * P, 0:512], in_=y[:, 0:512]
                    )
                    nc.sync.dma_start(
                        out=d["out"][st * P : (st + 1) * P, 512:D], in_=y[:, 512:D]
                    )


def kernel(**inputs):
    inputs = {k: np.asarray(v) for k, v in inputs.items()}
    if "nc" not in _CACHE:
        _CACHE["nc"] = build_bass()
    nc = _CACHE["nc"]

    consts = _prep_weights(inputs)
    base = {}
    for k, v in consts.items():
        if k in F8_NAMES:
            base[k] = np.ascontiguousarray(v).astype(ml_dtypes.float8_e4m3)
        elif k in BF16_NAMES:
            base[k] = np.ascontiguousarray(v).astype(ml_dtypes.bfloat16)
        else:
            base[k] = np.ascontiguousarray(v).astype(np.float32)

    in_maps = []
    for core in range(B):
        m = dict(base)
        m["x"] = np.ascontiguousarray(inputs["x"][core]).astype(np.float32)
        m["cT"] = _to_pmaj(inputs["c"][core]).astype(np.float32)
        in_maps.append(m)

    res = run_bass_kernel_spmd(
        nc, in_maps, core_ids=list(range(B)), **_CACHE.get("run_kwargs", {})
    )
    _CACHE["last_results"] = res
    return np.stack([res.results[i]["out"] for i in range(B)], axis=0)


if __name__ == "__main__":
    build_bass()
    print("built ok")
